# revision 9
# baseline (speedup 1.0000x reference)
"""Trainium2 Bass kernel for nn_CA_82300163326040.

Cross-attention between X and Y with softmax over the BATCH axis (torch
legacy dim=0). B=8, N=2048, D=512, f32.

Sharding: data-parallel over batch, one batch per NeuronCore (8 cores).
The batch-axis softmax couples cores: each core computes its local
exp-scores E1=exp(Q1.K2^T*s), E2=exp(Q2.K1^T*s) ([2048,2048]) and the
denominators Z = sum_b E via 8-core bf16 AllReduces (fp8 AR measured
too lossy: RDH requantizes partial sums at every hop).

v2 over the 579us baseline:
  - A@V matmuls run fp8 DoubleRow (2x PE rate) on the CENTERED attention
    matrix Ahat = A - 1/8. A clusters tightly around 1/8 (softmax over 8
    batches), so Ahat has ~3x smaller dynamic range -> fp8 quantization
    noise of both Ahat and V is suppressed. The exact rank-1 correction
    (1/8)*colsum(V1+V2) is computed on HOST in f64 (colsum(V) =
    colsum(X)@Wv^T + N*bv) and folded into the residual XYF.
  - 1/sqrt(D) applied via the exp ACT `scale` (not folded into Wq),
    keeping Q/K at natural magnitude.
  - Schedule: Q1,K2 proj -> E1 chunks (AR E1 halves start ~90us vs 143),
    K1,Q2 -> E2 chunks (AR E2 halves), V projections (fp8 out) fill the
    AR window, then U passes keyed to Z arrival.
  - E blocked [ch, p, (mt c)]: one contiguous 2.1MB DMA per chunk
    (16KB/partition rows) instead of 16 tile writes.
  - Phase C per-chunk chain spread across engines: Z->f32 copy (gpsimd),
    reciprocal_approx_fast (DVE), T = E - 0.125*Z fused scalar_tensor_
    tensor (DVE, 16-bit), Ahat = T*R -> fp8 (gpsimd). U1 partial sums
    HELD OPEN in PSUM; U2 appends into the same accumulation group and
    evicts adding the (X+Y+corr)^T residual.
"""

import numpy as np

import concourse.bass as bass
import concourse.mybir as mybir
import concourse.tile as tile
from concourse import bacc
from concourse.bass_utils import run_bass_kernel_spmd

P = 128
N = 2048  # sequence length
D = 512  # model dim
NCORES = 8
DT = D // P  # 4 feature tiles
NT = N // P  # 16 sequence tiles
CH = 512  # n-chunk (free dim of all matmuls)
NCH = N // CH  # 4 chunks
QT = 2  # mt-tiles per phase-C elementwise slice
SCALE = float(1.0 / np.sqrt(D))

F32 = mybir.dt.float32
BF16 = mybir.dt.bfloat16
F8 = mybir.dt.float8e4

_CACHE = {}


def build(ar_dtype=BF16):
    nc = bacc.Bacc("TRN2", target_bir_lowering=False, debug=False, num_devices=NCORES)

    xtb = nc.declare_dram_parameter("XTB", [P, DT, N], BF16, isOutput=False)
    ytb = nc.declare_dram_parameter("YTB", [P, DT, N], BF16, isOutput=False)
    # residual (X+Y)^T + (1/8)(colsum V1 + colsum V2), blocked [dt, ch, p, c]
    xyf = nc.declare_dram_parameter("XYF", [DT, NCH, P, CH], F32, isOutput=False)
    w_q1 = nc.declare_dram_parameter("WQ1T", [P, DT, D], BF16, isOutput=False)
    w_k1 = nc.declare_dram_parameter("WK1T", [P, DT, D], BF16, isOutput=False)
    w_v1 = nc.declare_dram_parameter("WV1T", [P, DT, D], BF16, isOutput=False)
    w_q2 = nc.declare_dram_parameter("WQ2T", [P, DT, D], BF16, isOutput=False)
    w_k2 = nc.declare_dram_parameter("WK2T", [P, DT, D], BF16, isOutput=False)
    w_v2 = nc.declare_dram_parameter("WV2T", [P, DT, D], BF16, isOutput=False)
    b_q1 = nc.declare_dram_parameter("BQ1", [P, DT], F32, isOutput=False)
    b_k1 = nc.declare_dram_parameter("BK1", [P, DT], F32, isOutput=False)
    b_q2 = nc.declare_dram_parameter("BQ2", [P, DT], F32, isOutput=False)
    b_k2 = nc.declare_dram_parameter("BK2", [P, DT], F32, isOutput=False)
    b_v1 = nc.declare_dram_parameter("BV1", [P, D], F32, isOutput=False)
    b_v2 = nc.declare_dram_parameter("BV2", [P, D], F32, isOutput=False)

    out = nc.declare_dram_parameter("OT", [DT, NCH, P, CH], F32, isOutput=True)

    NTCH = NT * CH  # 8192 free elems per partition per chunk

    with tile.TileContext(nc) as tc:
        with (
            tc.tile_pool(name="xy", bufs=2) as p_xy,
            tc.tile_pool(name="qk", bufs=2) as p_qk,
            tc.tile_pool(name="w", bufs=2) as p_w,
            tc.tile_pool(name="bias", bufs=1) as p_bias,
            tc.tile_pool(name="v", bufs=2) as p_v,
            tc.tile_pool(name="est", bufs=2) as p_est,
            tc.tile_pool(name="eb", bufs=2) as p_eb,
            tc.tile_pool(name="zb", bufs=2) as p_zb,
            tc.tile_pool(name="zf", bufs=4) as p_zf,
            tc.tile_pool(name="ah", bufs=2) as p_ah,
            tc.tile_pool(name="small", bufs=4) as p_small,
            tc.tile_pool(name="ps", bufs=8, space="PSUM") as p_ps,
            tc.tile_pool(name="dram", bufs=1, space="DRAM") as p_dram,
        ):
            # DRAM intermediates, blocked [ch, p, mt, c] (p-major: 16KB rows)
            e1_d = p_dram.tile([NCH, P, NT, CH], ar_dtype, tag="e1")
            e2_d = p_dram.tile([NCH, P, NT, CH], ar_dtype, tag="e2")
            z1_h = [
                p_dram.tile([2, P, NT, CH], ar_dtype, tag=f"z1{h}",
                            addr_space="Shared", name=f"z1{h}")
                for h in range(2)
            ]
            z2_h = [
                p_dram.tile([2, P, NT, CH], ar_dtype, tag=f"z2{h}",
                            addr_space="Shared", name=f"z2{h}")
                for h in range(2)
            ]

            # resident loads
            xt_sb = p_xy.tile([P, DT, N], BF16, tag="xy", name="xt")
            yt_sb = p_xy.tile([P, DT, N], BF16, tag="xy", name="yt")
            nc.sync.dma_start(xt_sb[:], xtb[:])
            nc.sync.dma_start(yt_sb[:], ytb[:])

            bq1_sb = p_bias.tile([P, DT], F32, tag="bq1")
            bk1_sb = p_bias.tile([P, DT], F32, tag="bk1")
            bq2_sb = p_bias.tile([P, DT], F32, tag="bq2")
            bk2_sb = p_bias.tile([P, DT], F32, tag="bk2")
            bv1_sb = p_bias.tile([P, D], F32, tag="bv1")
            bv2_sb = p_bias.tile([P, D], F32, tag="bv2")
            nc.sync.dma_start(bq1_sb[:], b_q1[:])
            nc.sync.dma_start(bk1_sb[:], b_k1[:])
            nc.sync.dma_start(bq2_sb[:], b_q2[:])
            nc.sync.dma_start(bk2_sb[:], b_k2[:])
            nc.sync.dma_start(bv1_sb[:], b_v1[:])
            nc.sync.dma_start(bv2_sb[:], b_v2[:])

            def load_w(wp):
                w_sb = p_w.tile([P, DT, D], BF16, tag="w")
                nc.sync.dma_start(w_sb[:], wp[:])
                return w_sb

            def proj_T(w_sb, src_sb, bias_sb, name):
                """out[e, n] = sum_d W[e,d] src[n,d] + b[e], e-major bf16."""
                o_sb = p_qk.tile([P, DT, N], BF16, tag="qk", name=name)
                for eo in range(DT):
                    for ch in range(NCH):
                        ps = p_ps.tile([P, CH], F32, tag="ps")
                        for do in range(DT):
                            nc.tensor.matmul(
                                ps[:],
                                w_sb[:, do, eo * P : (eo + 1) * P],
                                src_sb[:, do, ch * CH : (ch + 1) * CH],
                                start=(do == 0),
                                stop=(do == DT - 1),
                            )
                        nc.scalar.activation(
                            o_sb[:, eo, ch * CH : (ch + 1) * CH],
                            ps[:],
                            mybir.ActivationFunctionType.Identity,
                            bias=bias_sb[:, eo : eo + 1],
                        )
                return o_sb

            def proj_V(w_sb, src_sb, bias_sb, name):
                """out[m, e] = sum_d src[m,d] W[e,d] + b[e], m-major fp8."""
                o_sb = p_v.tile([P, NT, D], F8, tag="v", name=name)
                for mt in range(NT):
                    ps = p_ps.tile([P, CH], F32, tag="ps")
                    for do in range(DT):
                        nc.tensor.matmul(
                            ps[:],
                            src_sb[:, do, mt * P : (mt + 1) * P],
                            w_sb[:, do, :],
                            start=(do == 0),
                            stop=(do == DT - 1),
                        )
                    nc.vector.tensor_add(out=o_sb[:, mt, :], in0=ps[:], in1=bias_sb[:])
                return o_sb

            def scores_chunk(kt_sb, qt_sb, e_dram, ch):
                """E[ch] = exp(scale * K^T Q) -> bf16 chunk -> DRAM (one DMA)."""
                HT = NT // 2
                for hf in range(2):
                    e_sb = p_est.tile([P, HT, CH], ar_dtype, tag="est")
                    for mi in range(HT):
                        mt = hf * HT + mi
                        ps = p_ps.tile([P, CH], F32, tag="ps")
                        for eo in range(DT):
                            nc.tensor.matmul(
                                ps[:],
                                kt_sb[:, eo, mt * P : (mt + 1) * P],
                                qt_sb[:, eo, ch * CH : (ch + 1) * CH],
                                start=(eo == 0),
                                stop=(eo == DT - 1),
                            )
                        nc.scalar.activation(
                            e_sb[:, mi, :], ps[:],
                            mybir.ActivationFunctionType.Exp,
                            scale=SCALE,
                        )
                    nc.sync.dma_start(
                        e_dram[ch, :, hf * HT : (hf + 1) * HT, :], e_sb[:]
                    )

            def ar_half(e_d, z_halves, h):
                nc.gpsimd.collective_compute(
                    "AllReduce",
                    mybir.AluOpType.add,
                    replica_groups=[list(range(NCORES))],
                    ins=[e_d[2 * h : 2 * h + 2].opt()],
                    outs=[z_halves[h][:].opt()],
                )

            # ======== phase A1: Q1, K2 ========
            w_sb = load_w(w_q1)
            q1t = proj_T(w_sb, xt_sb, bq1_sb, "q1t")
            w_sb = load_w(w_k2)
            k2t = proj_T(w_sb, yt_sb, bk2_sb, "k2t")

            # ======== phase B1: E1 + its ARs ========
            scores_chunk(k2t, q1t, e1_d, 0)
            scores_chunk(k2t, q1t, e1_d, 1)
            ar_half(e1_d, z1_h, 0)
            scores_chunk(k2t, q1t, e1_d, 2)
            scores_chunk(k2t, q1t, e1_d, 3)
            ar_half(e1_d, z1_h, 1)

            # ======== phase A2: K1, Q2 (q1t/k2t slots recycled later) ========
            w_sb = load_w(w_k1)
            k1t = proj_T(w_sb, xt_sb, bk1_sb, "k1t")
            w_sb = load_w(w_q2)
            q2t = proj_T(w_sb, yt_sb, bq2_sb, "q2t")

            # ======== phase B2: E2 + its ARs ========
            scores_chunk(k1t, q2t, e2_d, 0)
            scores_chunk(k1t, q2t, e2_d, 1)
            ar_half(e2_d, z2_h, 0)
            scores_chunk(k1t, q2t, e2_d, 2)
            scores_chunk(k1t, q2t, e2_d, 3)
            ar_half(e2_d, z2_h, 1)

            # ======== phase A3: V projections (fp8 out), fills AR window ====
            w_sb = load_w(w_v2)
            v2 = proj_V(w_sb, yt_sb, bv2_sb, "v2")
            w_sb = load_w(w_v1)
            v1 = proj_V(w_sb, xt_sb, bv1_sb, "v1")

            # ======== phase C ========
            def make_ahat(e_d, z_src, ch, name):
                """Ahat[:, mt, c] = E/Z - 1/8 for chunk ch; fp8 [P, NT, CH]."""
                HT = NT // 2
                a_sb = p_ah.tile([P, NT, CH], F8, tag="ah", name=f"a{name}")
                for hf in range(2):
                    hsl = slice(hf * HT, (hf + 1) * HT)
                    eb = p_eb.tile([P, HT, CH], ar_dtype, tag="eb", name=f"eb{name}")
                    nc.sync.dma_start(eb[:], e_d[ch, :, hsl, :])
                    zb = p_zb.tile([P, HT, CH], ar_dtype, tag="zb", name=f"zb{name}")
                    nc.sync.dma_start(zb[:], z_src(ch)[:, hsl, :])
                    for q in range(HT // QT):
                        msl = slice(q * QT, (q + 1) * QT)
                        asl = slice(hf * HT + q * QT, hf * HT + (q + 1) * QT)
                        zf = p_zf.tile([P, QT, CH], F32, tag="zf", name=f"zf{name}")
                        nc.gpsimd.tensor_copy(out=zf[:], in_=zb[:, msl, :])
                        rz = p_zf.tile([P, QT, CH], F32, tag="zf", name=f"rz{name}")
                        nc.vector.reciprocal_approx_fast(out=rz[:], in_=zf[:])
                        tq = p_zf.tile([P, QT, CH], BF16, tag="zf", name=f"t{name}")
                        nc.vector.scalar_tensor_tensor(
                            out=tq[:],
                            in0=zb[:, msl, :],
                            scalar=-0.125,
                            in1=eb[:, msl, :],
                            op0=mybir.AluOpType.mult,
                            op1=mybir.AluOpType.add,
                        )
                        nc.gpsimd.tensor_mul(
                            out=a_sb[:, asl, :], in0=tq[:], in1=rz[:]
                        )
                return a_sb

            def z1_src(ch):
                return z1_h[ch // 2][ch % 2]

            def z2_src(ch):
                return z2_h[ch // 2][ch % 2]

            ps_held = {}

            def u1_pass(ch):
                a1 = make_ahat(e1_d, z1_src, ch, f"1{ch}")
                tiles = []
                for dt in range(DT):
                    dsl = slice(dt * P, (dt + 1) * P)
                    ps = p_ps.tile([P, CH], F32, tag="ps")
                    for tp in range(NT // 2):
                        nc.tensor.matmul(
                            ps[:],
                            v2[:, 2 * tp : 2 * tp + 2, dsl],
                            a1[:, 2 * tp : 2 * tp + 2, :],
                            start=(tp == 0),
                            stop=False,
                            perf_mode=mybir.MatmulPerfMode.DoubleRow,
                        )
                    tiles.append(ps)
                ps_held[ch] = tiles

            def u2_pass(ch):
                a2 = make_ahat(e2_d, z2_src, ch, f"2{ch}")
                for dt in range(DT):
                    dsl = slice(dt * P, (dt + 1) * P)
                    ps = ps_held[ch][dt]
                    for tp in range(NT // 2):
                        nc.tensor.matmul(
                            ps[:],
                            v1[:, 2 * tp : 2 * tp + 2, dsl],
                            a2[:, 2 * tp : 2 * tp + 2, :],
                            start=False,
                            stop=(tp == NT // 2 - 1),
                            perf_mode=mybir.MatmulPerfMode.DoubleRow,
                        )
                    xyres = p_small.tile([P, CH], F32, tag="xyres")
                    nc.scalar.dma_start(xyres[:], xyf[dt, ch])
                    ot = p_small.tile([P, CH], F32, tag="ot")
                    nc.vector.tensor_add(out=ot[:], in0=ps[:], in1=xyres[:])
                    nc.scalar.dma_start(out[dt, ch], ot[:])

            u1_pass(0)
            u1_pass(1)
            u2_pass(0)
            u1_pass(2)
            u2_pass(1)
            u1_pass(3)
            u2_pass(2)
            u2_pass(3)

    nc.compile()
    return nc


def _pmajor(a, inner):
    """[O*P, F] -> [P, O, F] partition-major."""
    o = a.shape[0] // inner
    return np.ascontiguousarray(a.reshape(o, inner, a.shape[1]).transpose(1, 0, 2))


def _blocked(a):
    """[D, N] -> [DT, NCH, P, CH] blocked."""
    return np.ascontiguousarray(a.reshape(DT, P, NCH, CH).transpose(0, 2, 1, 3))


def _prep_inputs(inputs):
    import ml_dtypes

    X = np.asarray(inputs["X"], dtype=np.float32)
    Y = np.asarray(inputs["Y"], dtype=np.float32)

    def wT(name):
        w = np.asarray(inputs[f"W_{name}"], dtype=np.float32)
        return _pmajor(w.T.astype(ml_dtypes.bfloat16), P)

    def bstripe(name):
        b = np.asarray(inputs[f"b_{name}"], dtype=np.float32)
        return np.ascontiguousarray(b.reshape(DT, P).T)

    def bbcast(name):
        b = np.asarray(inputs[f"b_{name}"], dtype=np.float32)
        return np.ascontiguousarray(np.broadcast_to(b, (P, D)))

    shared = {
        "WQ1T": wT("xq"),
        "WK1T": wT("xk"),
        "WV1T": wT("xv"),
        "WQ2T": wT("yq"),
        "WK2T": wT("yk"),
        "WV2T": wT("yv"),
        "BQ1": bstripe("xq"),
        "BK1": bstripe("xk"),
        "BQ2": bstripe("yq"),
        "BK2": bstripe("yk"),
        "BV1": bbcast("xv"),
        "BV2": bbcast("yv"),
    }
    # host-exact centered-A correction: (1/8) * colsum_m(V1[m,:] + V2[m,:])
    Wxv = np.asarray(inputs["W_xv"], np.float64)
    Wyv = np.asarray(inputs["W_yv"], np.float64)
    bxv = np.asarray(inputs["b_xv"], np.float64)
    byv = np.asarray(inputs["b_yv"], np.float64)
    xs = X.astype(np.float64).sum(axis=1)  # [B, D] colsum of X over tokens
    ys = Y.astype(np.float64).sum(axis=1)
    cs1 = xs @ Wxv.T + N * bxv  # [B, D] = colsum_m V1
    cs2 = ys @ Wyv.T + N * byv
    corr = ((cs1 + cs2) / 8.0).astype(np.float32)  # [B, D]

    in_maps = []
    for c in range(NCORES):
        xt = np.ascontiguousarray(X[c].T)
        yt = np.ascontiguousarray(Y[c].T)
        m = dict(shared)
        m["XYF"] = _blocked((xt + yt) + corr[c][:, None])
        m["XTB"] = _pmajor(xt.astype(ml_dtypes.bfloat16), P)
        m["YTB"] = _pmajor(yt.astype(ml_dtypes.bfloat16), P)
        in_maps.append(m)
    return in_maps


def _unblock(ot):
    """[DT, NCH, P, CH] -> [N, D] (transposed back)."""
    return ot.transpose(0, 2, 1, 3).reshape(D, N).T


def kernel(**inputs):
    if "nc" not in _CACHE:
        _CACHE["nc"] = build()
    nc = _CACHE["nc"]
    in_maps = _prep_inputs(inputs)
    res = run_bass_kernel_spmd(
        nc, in_maps, core_ids=list(range(NCORES)), **_CACHE.get("run_kwargs", {})
    )
    _CACHE["last_result"] = res
    out = np.stack(
        [np.ascontiguousarray(_unblock(res.results[c]["OT"])) for c in range(NCORES)]
    )
    return out.astype(np.float32)


# revision 15
# speedup vs baseline: 1.2139x; 1.2139x over previous
"""Trainium2 Bass kernel for nn_CA_82300163326040.

Cross-attention between X and Y with softmax over the BATCH axis (torch
legacy dim=0). B=8, N=2048, D=512, f32.

Sharding: data-parallel over batch, one batch per NeuronCore (8 cores).
The batch-axis softmax couples cores: each core computes its local
exp-scores E1=exp(Q1.K2^T*s), E2=exp(Q2.K1^T*s) ([2048,2048]) and the
denominators Z = sum_b E via 8-core bf16 AllReduces (fp8 AR measured
too lossy: RDH requantizes partial sums at every hop).

v2 over the 579us baseline:
  - A@V matmuls run fp8 DoubleRow (2x PE rate) on the CENTERED attention
    matrix Ahat = A - 1/8. A clusters tightly around 1/8 (softmax over 8
    batches), so Ahat has ~3x smaller dynamic range -> fp8 quantization
    noise of both Ahat and V is suppressed. The exact rank-1 correction
    (1/8)*colsum(V1+V2) is computed on HOST in f64 (colsum(V) =
    colsum(X)@Wv^T + N*bv) and folded into the residual XYF.
  - 1/sqrt(D) applied via the exp ACT `scale` (not folded into Wq),
    keeping Q/K at natural magnitude.
  - Schedule: Q1,K2 proj -> E1 chunks (AR E1 halves start ~90us vs 143),
    K1,Q2 -> E2 chunks (AR E2 halves), V projections (fp8 out) fill the
    AR window, then U passes keyed to Z arrival.
  - E blocked [ch, p, (mt c)]: one contiguous 2.1MB DMA per chunk
    (16KB/partition rows) instead of 16 tile writes.
  - Phase C per-chunk chain spread across engines: Z->f32 copy (gpsimd),
    reciprocal_approx_fast (DVE), T = E - 0.125*Z fused scalar_tensor_
    tensor (DVE, 16-bit), Ahat = T*R -> fp8 (gpsimd). U1 partial sums
    HELD OPEN in PSUM; U2 appends into the same accumulation group and
    evicts adding the (X+Y+corr)^T residual.
"""

import numpy as np

import concourse.bass as bass
import concourse.mybir as mybir
import concourse.tile as tile
from concourse import bacc
from concourse.bass_utils import run_bass_kernel_spmd

P = 128
N = 2048  # sequence length
D = 512  # model dim
NCORES = 8
DT = D // P  # 4 feature tiles
NT = N // P  # 16 sequence tiles
CH = 512  # n-chunk (free dim of all matmuls)
NCH = N // CH  # 4 chunks
QT = 2  # mt-tiles per phase-C elementwise slice
SCALE = float(1.0 / np.sqrt(D))

F32 = mybir.dt.float32
BF16 = mybir.dt.bfloat16
F8 = mybir.dt.float8e4

_CACHE = {}


def build(ar_dtype=BF16):
    nc = bacc.Bacc("TRN2", target_bir_lowering=False, debug=False, num_devices=NCORES)

    xtb = nc.declare_dram_parameter("XTB", [P, DT, N], BF16, isOutput=False)
    ytb = nc.declare_dram_parameter("YTB", [P, DT, N], BF16, isOutput=False)
    # residual (X+Y)^T + (1/8)(colsum V1 + colsum V2), blocked [dt, ch, p, c]
    xyf = nc.declare_dram_parameter("XYF", [DT, NCH, P, CH], F32, isOutput=False)
    w_q1 = nc.declare_dram_parameter("WQ1T", [P, DT, D], BF16, isOutput=False)
    w_k1 = nc.declare_dram_parameter("WK1T", [P, DT, D], BF16, isOutput=False)
    w_v1 = nc.declare_dram_parameter("WV1T", [P, DT, D], BF16, isOutput=False)
    w_q2 = nc.declare_dram_parameter("WQ2T", [P, DT, D], BF16, isOutput=False)
    w_k2 = nc.declare_dram_parameter("WK2T", [P, DT, D], BF16, isOutput=False)
    w_v2 = nc.declare_dram_parameter("WV2T", [P, DT, D], BF16, isOutput=False)
    b_q1 = nc.declare_dram_parameter("BQ1", [P, DT], F32, isOutput=False)
    b_k1 = nc.declare_dram_parameter("BK1", [P, DT], F32, isOutput=False)
    b_q2 = nc.declare_dram_parameter("BQ2", [P, DT], F32, isOutput=False)
    b_k2 = nc.declare_dram_parameter("BK2", [P, DT], F32, isOutput=False)
    b_v1 = nc.declare_dram_parameter("BV1", [P, D], F32, isOutput=False)
    b_v2 = nc.declare_dram_parameter("BV2", [P, D], F32, isOutput=False)

    out = nc.declare_dram_parameter("OT", [DT, NCH, P, CH], F32, isOutput=True)

    NTCH = NT * CH  # 8192 free elems per partition per chunk

    with tile.TileContext(nc) as tc:
        with (
            tc.tile_pool(name="xy", bufs=2) as p_xy,
            tc.tile_pool(name="qk", bufs=2) as p_qk,
            tc.tile_pool(name="w", bufs=2) as p_w,
            tc.tile_pool(name="bias", bufs=1) as p_bias,
            tc.tile_pool(name="v", bufs=2) as p_v,
            tc.tile_pool(name="est", bufs=2) as p_est,
            tc.tile_pool(name="eb", bufs=2) as p_eb,
            tc.tile_pool(name="zb", bufs=2) as p_zb,
            tc.tile_pool(name="zf", bufs=4) as p_zf,
            tc.tile_pool(name="ah", bufs=2) as p_ah,
            tc.tile_pool(name="small", bufs=4) as p_small,
            tc.tile_pool(name="ps", bufs=8, space="PSUM") as p_ps,
            tc.tile_pool(name="dram", bufs=1, space="DRAM") as p_dram,
        ):
            # DRAM intermediates, blocked [ch, p, mt, c] (p-major: 16KB rows)
            e1_d = p_dram.tile([NCH, P, NT, CH], ar_dtype, tag="e1")
            e2_d = p_dram.tile([NCH, P, NT, CH], ar_dtype, tag="e2")
            z1_h = [
                p_dram.tile([2, P, NT, CH], ar_dtype, tag=f"z1{h}",
                            addr_space="Shared", name=f"z1{h}")
                for h in range(2)
            ]
            z2_h0 = p_dram.tile([2, P, NT, CH], ar_dtype, tag="z2h0",
                                addr_space="Shared", name="z2h0")
            z2_q = [
                p_dram.tile([1, P, NT, CH], ar_dtype, tag=f"z2q{i}",
                            addr_space="Shared", name=f"z2q{i}")
                for i in range(2)
            ]

            # resident loads
            xt_sb = p_xy.tile([P, DT, N], BF16, tag="xy", name="xt")
            yt_sb = p_xy.tile([P, DT, N], BF16, tag="xy", name="yt")
            nc.sync.dma_start(xt_sb[:], xtb[:])
            nc.sync.dma_start(yt_sb[:], ytb[:])

            bq1_sb = p_bias.tile([P, DT], F32, tag="bq1")
            bk1_sb = p_bias.tile([P, DT], F32, tag="bk1")
            bq2_sb = p_bias.tile([P, DT], F32, tag="bq2")
            bk2_sb = p_bias.tile([P, DT], F32, tag="bk2")
            bv1_sb = p_bias.tile([P, D], F32, tag="bv1")
            bv2_sb = p_bias.tile([P, D], F32, tag="bv2")
            nc.sync.dma_start(bq1_sb[:], b_q1[:])
            nc.sync.dma_start(bk1_sb[:], b_k1[:])
            nc.sync.dma_start(bq2_sb[:], b_q2[:])
            nc.sync.dma_start(bk2_sb[:], b_k2[:])
            nc.sync.dma_start(bv1_sb[:], b_v1[:])
            nc.sync.dma_start(bv2_sb[:], b_v2[:])

            def load_w(wp):
                w_sb = p_w.tile([P, DT, D], BF16, tag="w")
                nc.sync.dma_start(w_sb[:], wp[:])
                return w_sb

            def proj_T(w_sb, src_sb, bias_sb, name):
                """out[e, n] = sum_d W[e,d] src[n,d] + b[e], e-major bf16."""
                o_sb = p_qk.tile([P, DT, N], BF16, tag="qk", name=name)
                for eo in range(DT):
                    for ch in range(NCH):
                        ps = p_ps.tile([P, CH], F32, tag="ps")
                        for do in range(DT):
                            nc.tensor.matmul(
                                ps[:],
                                w_sb[:, do, eo * P : (eo + 1) * P],
                                src_sb[:, do, ch * CH : (ch + 1) * CH],
                                start=(do == 0),
                                stop=(do == DT - 1),
                            )
                        nc.scalar.activation(
                            o_sb[:, eo, ch * CH : (ch + 1) * CH],
                            ps[:],
                            mybir.ActivationFunctionType.Identity,
                            bias=bias_sb[:, eo : eo + 1],
                        )
                return o_sb

            def proj_T_chunk(w_sb, src_sb, bias_sb, o_sb, ch):
                for eo in range(DT):
                    ps = p_ps.tile([P, CH], F32, tag="ps")
                    for do in range(DT):
                        nc.tensor.matmul(
                            ps[:],
                            w_sb[:, do, eo * P : (eo + 1) * P],
                            src_sb[:, do, ch * CH : (ch + 1) * CH],
                            start=(do == 0),
                            stop=(do == DT - 1),
                        )
                    nc.scalar.activation(
                        o_sb[:, eo, ch * CH : (ch + 1) * CH],
                        ps[:],
                        mybir.ActivationFunctionType.Identity,
                        bias=bias_sb[:, eo : eo + 1],
                    )

            def proj_V(w_sb, src_sb, bias_sb, name):
                """out[m, e] = sum_d src[m,d] W[e,d] + b[e], m-major fp8."""
                o_sb = p_v.tile([P, NT, D], F8, tag="v", name=name)
                for mt in range(NT):
                    ps = p_ps.tile([P, CH], F32, tag="ps")
                    for do in range(DT):
                        nc.tensor.matmul(
                            ps[:],
                            src_sb[:, do, mt * P : (mt + 1) * P],
                            w_sb[:, do, :],
                            start=(do == 0),
                            stop=(do == DT - 1),
                        )
                    nc.vector.tensor_add(out=o_sb[:, mt, :], in0=ps[:], in1=bias_sb[:])
                return o_sb

            def scores_chunk(kt_sb, qt_sb, e_dram, ch):
                """E[ch] = exp(scale * K^T Q) -> bf16 chunk -> DRAM (one DMA)."""
                HT = NT // 2
                for hf in range(2):
                    e_sb = p_est.tile([P, HT, CH], ar_dtype, tag="est")
                    for mi in range(HT):
                        mt = hf * HT + mi
                        ps = p_ps.tile([P, CH], F32, tag="ps")
                        for eo in range(DT):
                            nc.tensor.matmul(
                                ps[:],
                                kt_sb[:, eo, mt * P : (mt + 1) * P],
                                qt_sb[:, eo, ch * CH : (ch + 1) * CH],
                                start=(eo == 0),
                                stop=(eo == DT - 1),
                            )
                        nc.scalar.activation(
                            e_sb[:, mi, :], ps[:],
                            mybir.ActivationFunctionType.Exp,
                            scale=SCALE,
                        )
                    nc.sync.dma_start(
                        e_dram[ch, :, hf * HT : (hf + 1) * HT, :], e_sb[:]
                    )

            def ar_half(e_d, z_halves, h):
                nc.gpsimd.collective_compute(
                    "AllReduce",
                    mybir.AluOpType.add,
                    replica_groups=[list(range(NCORES))],
                    ins=[e_d[2 * h : 2 * h + 2].opt()],
                    outs=[z_halves[h][:].opt()],
                )

            # ==== phase A1/B1: K2 full, then Q1 chunks fused with E1 ====
            w_k2sb = load_w(w_k2)
            k2t = proj_T(w_k2sb, yt_sb, bk2_sb, "k2t")
            w_q1sb = load_w(w_q1)
            q1t = p_qk.tile([P, DT, N], BF16, tag="qk", name="q1t")
            for ch in range(NCH):
                proj_T_chunk(w_q1sb, xt_sb, bq1_sb, q1t, ch)
                scores_chunk(k2t, q1t, e1_d, ch)
                if ch == 1:
                    ar_half(e1_d, z1_h, 0)
            ar_half(e1_d, z1_h, 1)

            # ==== phase A2/B2: K1 full, then Q2 chunks fused with E2 ====
            w_k1sb = load_w(w_k1)
            k1t = proj_T(w_k1sb, xt_sb, bk1_sb, "k1t")
            w_q2sb = load_w(w_q2)
            q2t = p_qk.tile([P, DT, N], BF16, tag="qk", name="q2t")
            for ch in range(NCH):
                proj_T_chunk(w_q2sb, yt_sb, bq2_sb, q2t, ch)
                scores_chunk(k1t, q2t, e2_d, ch)
                if ch == 1:
                    ar_half(e2_d, [z2_h0], 0)
            # last half split into two quarter-ARs so the final chunk's
            # Z lands sooner and the tail chain shortens
            nc.gpsimd.collective_compute(
                "AllReduce", mybir.AluOpType.add,
                replica_groups=[list(range(NCORES))],
                ins=[e2_d[2:3].opt()], outs=[z2_q[0][:].opt()],
            )
            nc.gpsimd.collective_compute(
                "AllReduce", mybir.AluOpType.add,
                replica_groups=[list(range(NCORES))],
                ins=[e2_d[3:4].opt()], outs=[z2_q[1][:].opt()],
            )

            # ======== phase A3: V projections (fp8 out), fills AR window ====
            w_sb = load_w(w_v2)
            v2 = proj_V(w_sb, yt_sb, bv2_sb, "v2")
            w_sb = load_w(w_v1)
            v1 = proj_V(w_sb, xt_sb, bv1_sb, "v1")

            # ======== phase C ========
            def make_ahat(e_d, z_src, ch, name):
                """Ahat[:, mt, c] = E/Z - 1/8 for chunk ch; fp8 [P, NT, CH]."""
                HT = NT // 2
                a_sb = p_ah.tile([P, NT, CH], F8, tag="ah", name=f"a{name}")
                for hf in range(2):
                    hsl = slice(hf * HT, (hf + 1) * HT)
                    eb = p_eb.tile([P, HT, CH], ar_dtype, tag="eb", name=f"eb{name}")
                    nc.sync.dma_start(eb[:], e_d[ch, :, hsl, :])
                    zb = p_zb.tile([P, HT, CH], ar_dtype, tag="zb", name=f"zb{name}")
                    nc.sync.dma_start(zb[:], z_src(ch)[:, hsl, :])
                    for q in range(HT // QT):
                        msl = slice(q * QT, (q + 1) * QT)
                        asl = slice(hf * HT + q * QT, hf * HT + (q + 1) * QT)
                        tq = p_zf.tile([P, QT, CH], BF16, tag="zf", name=f"t{name}")
                        nc.vector.scalar_tensor_tensor(
                            out=tq[:],
                            in0=zb[:, msl, :],
                            scalar=-0.125,
                            in1=eb[:, msl, :],
                            op0=mybir.AluOpType.mult,
                            op1=mybir.AluOpType.add,
                        )
                        zf = p_zf.tile([P, QT, CH], F32, tag="zf", name=f"zf{name}")
                        nc.scalar.activation(
                            zf[:], zb[:, msl, :],
                            mybir.ActivationFunctionType.Copy,
                        )
                        rz = p_zf.tile([P, QT, CH], F32, tag="zf", name=f"rz{name}")
                        nc.vector.reciprocal_approx_fast(out=rz[:], in_=zf[:])
                        nc.vector.tensor_mul(
                            out=a_sb[:, asl, :], in0=tq[:], in1=rz[:]
                        )
                return a_sb

            def z1_src(ch):
                return z1_h[ch // 2][ch % 2]

            def z2_src(ch):
                if ch < 2:
                    return z2_h0[ch]
                return z2_q[ch - 2][0]

            ps_held = {}

            def u1_pass(ch):
                a1 = make_ahat(e1_d, z1_src, ch, f"1{ch}")
                tiles = []
                for dt in range(DT):
                    dsl = slice(dt * P, (dt + 1) * P)
                    ps = p_ps.tile([P, CH], F32, tag="ps")
                    for tp in range(NT // 2):
                        nc.tensor.matmul(
                            ps[:],
                            v2[:, 2 * tp : 2 * tp + 2, dsl],
                            a1[:, 2 * tp : 2 * tp + 2, :],
                            start=(tp == 0),
                            stop=False,
                            perf_mode=mybir.MatmulPerfMode.DoubleRow,
                        )
                    tiles.append(ps)
                ps_held[ch] = tiles

            def u2_pass(ch):
                a2 = make_ahat(e2_d, z2_src, ch, f"2{ch}")
                for dt in range(DT):
                    dsl = slice(dt * P, (dt + 1) * P)
                    ps = ps_held[ch][dt]
                    for tp in range(NT // 2):
                        nc.tensor.matmul(
                            ps[:],
                            v1[:, 2 * tp : 2 * tp + 2, dsl],
                            a2[:, 2 * tp : 2 * tp + 2, :],
                            start=False,
                            stop=(tp == NT // 2 - 1),
                            perf_mode=mybir.MatmulPerfMode.DoubleRow,
                        )
                    xyres = p_small.tile([P, CH], F32, tag="xyres")
                    nc.scalar.dma_start(xyres[:], xyf[dt, ch])
                    ot = p_small.tile([P, CH], F32, tag="ot")
                    nc.vector.tensor_add(out=ot[:], in0=ps[:], in1=xyres[:])
                    nc.scalar.dma_start(out[dt, ch], ot[:])

            u1_pass(0)
            u1_pass(1)
            u2_pass(0)
            u1_pass(2)
            u2_pass(1)
            u1_pass(3)
            u2_pass(2)
            u2_pass(3)

    nc.compile()
    return nc


def _pmajor(a, inner):
    """[O*P, F] -> [P, O, F] partition-major."""
    o = a.shape[0] // inner
    return np.ascontiguousarray(a.reshape(o, inner, a.shape[1]).transpose(1, 0, 2))


def _blocked(a):
    """[D, N] -> [DT, NCH, P, CH] blocked."""
    return np.ascontiguousarray(a.reshape(DT, P, NCH, CH).transpose(0, 2, 1, 3))


def _prep_inputs(inputs):
    import ml_dtypes

    X = np.asarray(inputs["X"], dtype=np.float32)
    Y = np.asarray(inputs["Y"], dtype=np.float32)

    def wT(name):
        w = np.asarray(inputs[f"W_{name}"], dtype=np.float32)
        return _pmajor(w.T.astype(ml_dtypes.bfloat16), P)

    def bstripe(name):
        b = np.asarray(inputs[f"b_{name}"], dtype=np.float32)
        return np.ascontiguousarray(b.reshape(DT, P).T)

    def bbcast(name):
        b = np.asarray(inputs[f"b_{name}"], dtype=np.float32)
        return np.ascontiguousarray(np.broadcast_to(b, (P, D)))

    shared = {
        "WQ1T": wT("xq"),
        "WK1T": wT("xk"),
        "WV1T": wT("xv"),
        "WQ2T": wT("yq"),
        "WK2T": wT("yk"),
        "WV2T": wT("yv"),
        "BQ1": bstripe("xq"),
        "BK1": bstripe("xk"),
        "BQ2": bstripe("yq"),
        "BK2": bstripe("yk"),
        "BV1": bbcast("xv"),
        "BV2": bbcast("yv"),
    }
    # host-exact centered-A correction: (1/8) * colsum_m(V1[m,:] + V2[m,:])
    Wxv = np.asarray(inputs["W_xv"], np.float64)
    Wyv = np.asarray(inputs["W_yv"], np.float64)
    bxv = np.asarray(inputs["b_xv"], np.float64)
    byv = np.asarray(inputs["b_yv"], np.float64)
    xs = X.astype(np.float64).sum(axis=1)  # [B, D] colsum of X over tokens
    ys = Y.astype(np.float64).sum(axis=1)
    cs1 = xs @ Wxv.T + N * bxv  # [B, D] = colsum_m V1
    cs2 = ys @ Wyv.T + N * byv
    corr = ((cs1 + cs2) / 8.0).astype(np.float32)  # [B, D]

    in_maps = []
    for c in range(NCORES):
        xt = np.ascontiguousarray(X[c].T)
        yt = np.ascontiguousarray(Y[c].T)
        m = dict(shared)
        m["XYF"] = _blocked((xt + yt) + corr[c][:, None])
        m["XTB"] = _pmajor(xt.astype(ml_dtypes.bfloat16), P)
        m["YTB"] = _pmajor(yt.astype(ml_dtypes.bfloat16), P)
        in_maps.append(m)
    return in_maps


def _unblock(ot):
    """[DT, NCH, P, CH] -> [N, D] (transposed back)."""
    return ot.transpose(0, 2, 1, 3).reshape(D, N).T


def kernel(**inputs):
    if "nc" not in _CACHE:
        _CACHE["nc"] = build()
    nc = _CACHE["nc"]
    in_maps = _prep_inputs(inputs)
    res = run_bass_kernel_spmd(
        nc, in_maps, core_ids=list(range(NCORES)), **_CACHE.get("run_kwargs", {})
    )
    _CACHE["last_result"] = res
    out = np.stack(
        [np.ascontiguousarray(_unblock(res.results[c]["OT"])) for c in range(NCORES)]
    )
    return out.astype(np.float32)


# revision 16
# speedup vs baseline: 1.2647x; 1.0418x over previous
"""Trainium2 Bass kernel for nn_CA_82300163326040.

Cross-attention between X and Y with softmax over the BATCH axis (torch
legacy dim=0). B=8, N=2048, D=512, f32.

Sharding: data-parallel over batch, one batch per NeuronCore (8 cores).
The batch-axis softmax couples cores: each core computes its local
exp-scores E1=exp(Q1.K2^T*s), E2=exp(Q2.K1^T*s) ([2048,2048]) and the
denominators Z = sum_b E via 8-core bf16 AllReduces (fp8 AR measured
too lossy: RDH requantizes partial sums at every hop).

v2 over the 579us baseline:
  - A@V matmuls run fp8 DoubleRow (2x PE rate) on the CENTERED attention
    matrix Ahat = A - 1/8. A clusters tightly around 1/8 (softmax over 8
    batches), so Ahat has ~3x smaller dynamic range -> fp8 quantization
    noise of both Ahat and V is suppressed. The exact rank-1 correction
    (1/8)*colsum(V1+V2) is computed on HOST in f64 (colsum(V) =
    colsum(X)@Wv^T + N*bv) and folded into the residual XYF.
  - 1/sqrt(D) applied via the exp ACT `scale` (not folded into Wq),
    keeping Q/K at natural magnitude.
  - Schedule: Q1,K2 proj -> E1 chunks (AR E1 halves start ~90us vs 143),
    K1,Q2 -> E2 chunks (AR E2 halves), V projections (fp8 out) fill the
    AR window, then U passes keyed to Z arrival.
  - E blocked [ch, p, (mt c)]: one contiguous 2.1MB DMA per chunk
    (16KB/partition rows) instead of 16 tile writes.
  - Phase C per-chunk chain spread across engines: Z->f32 copy (gpsimd),
    reciprocal_approx_fast (DVE), T = E - 0.125*Z fused scalar_tensor_
    tensor (DVE, 16-bit), Ahat = T*R -> fp8 (gpsimd). U1 partial sums
    HELD OPEN in PSUM; U2 appends into the same accumulation group and
    evicts adding the (X+Y+corr)^T residual.
"""

import numpy as np

import concourse.bass as bass
import concourse.mybir as mybir
import concourse.tile as tile
from concourse import bacc
from concourse.bass_utils import run_bass_kernel_spmd

P = 128
N = 2048  # sequence length
D = 512  # model dim
NCORES = 8
DT = D // P  # 4 feature tiles
NT = N // P  # 16 sequence tiles
CH = 512  # n-chunk (free dim of all matmuls)
NCH = N // CH  # 4 chunks
QT = 2  # mt-tiles per phase-C elementwise slice
SCALE = float(1.0 / np.sqrt(D))

F32 = mybir.dt.float32
BF16 = mybir.dt.bfloat16
F8 = mybir.dt.float8e4

_CACHE = {}


def build(ar_dtype=BF16):
    nc = bacc.Bacc("TRN2", target_bir_lowering=False, debug=False, num_devices=NCORES)

    xtb = nc.declare_dram_parameter("XTB", [P, DT, N], BF16, isOutput=False)
    ytb = nc.declare_dram_parameter("YTB", [P, DT, N], BF16, isOutput=False)
    # residual (X+Y)^T + (1/8)(colsum V1 + colsum V2), blocked [dt, ch, p, c]
    xyf = nc.declare_dram_parameter("XYF", [DT, NCH, P, CH], F32, isOutput=False)
    w_q1 = nc.declare_dram_parameter("WQ1T", [P, DT, D], BF16, isOutput=False)
    w_k1 = nc.declare_dram_parameter("WK1T", [P, DT, D], BF16, isOutput=False)
    w_v1 = nc.declare_dram_parameter("WV1T", [P, DT, D], BF16, isOutput=False)
    w_q2 = nc.declare_dram_parameter("WQ2T", [P, DT, D], BF16, isOutput=False)
    w_k2 = nc.declare_dram_parameter("WK2T", [P, DT, D], BF16, isOutput=False)
    w_v2 = nc.declare_dram_parameter("WV2T", [P, DT, D], BF16, isOutput=False)
    b_q1 = nc.declare_dram_parameter("BQ1", [P, DT], F32, isOutput=False)
    b_k1 = nc.declare_dram_parameter("BK1", [P, DT], F32, isOutput=False)
    b_q2 = nc.declare_dram_parameter("BQ2", [P, DT], F32, isOutput=False)
    b_k2 = nc.declare_dram_parameter("BK2", [P, DT], F32, isOutput=False)
    b_v1 = nc.declare_dram_parameter("BV1", [P, D], F32, isOutput=False)
    b_v2 = nc.declare_dram_parameter("BV2", [P, D], F32, isOutput=False)

    out = nc.declare_dram_parameter("OT", [DT, NCH, P, CH], F32, isOutput=True)

    NTCH = NT * CH  # 8192 free elems per partition per chunk

    with tile.TileContext(nc) as tc:
        with (
            tc.tile_pool(name="xy", bufs=2) as p_xy,
            tc.tile_pool(name="qk", bufs=2) as p_qk,
            tc.tile_pool(name="w", bufs=2) as p_w,
            tc.tile_pool(name="bias", bufs=1) as p_bias,
            tc.tile_pool(name="v", bufs=2) as p_v,
            tc.tile_pool(name="est", bufs=2) as p_est,
            tc.tile_pool(name="eb", bufs=2) as p_eb,
            tc.tile_pool(name="zb", bufs=2) as p_zb,
            tc.tile_pool(name="zf", bufs=2) as p_zf,
            tc.tile_pool(name="ah", bufs=2) as p_ah,
            tc.tile_pool(name="small", bufs=4) as p_small,
            tc.tile_pool(name="ps", bufs=8, space="PSUM") as p_ps,
            tc.tile_pool(name="dram", bufs=1, space="DRAM") as p_dram,
        ):
            # DRAM intermediates, blocked [ch, p, mt, c] (p-major: 16KB rows)
            e1_d = p_dram.tile([NCH, P, NT, CH], ar_dtype, tag="e1")
            e2_d = p_dram.tile([NCH, P, NT, CH], ar_dtype, tag="e2")
            z1_h = [
                p_dram.tile([2, P, NT, CH], ar_dtype, tag=f"z1{h}",
                            addr_space="Shared", name=f"z1{h}")
                for h in range(2)
            ]
            z2_h0 = p_dram.tile([2, P, NT, CH], ar_dtype, tag="z2h0",
                                addr_space="Shared", name="z2h0")
            z2_h1 = p_dram.tile([2, P, NT, CH], ar_dtype, tag="z2h1",
                                addr_space="Shared", name="z2h1")

            # resident loads
            xt_sb = p_xy.tile([P, DT, N], BF16, tag="xy", name="xt")
            yt_sb = p_xy.tile([P, DT, N], BF16, tag="xy", name="yt")
            nc.sync.dma_start(yt_sb[:], ytb[:])
            nc.sync.dma_start(xt_sb[:], xtb[:])

            bq1_sb = p_bias.tile([P, DT], F32, tag="bq1")
            bk1_sb = p_bias.tile([P, DT], F32, tag="bk1")
            bq2_sb = p_bias.tile([P, DT], F32, tag="bq2")
            bk2_sb = p_bias.tile([P, DT], F32, tag="bk2")
            bv1_sb = p_bias.tile([P, D], F32, tag="bv1")
            bv2_sb = p_bias.tile([P, D], F32, tag="bv2")
            nc.sync.dma_start(bq1_sb[:], b_q1[:])
            nc.sync.dma_start(bk1_sb[:], b_k1[:])
            nc.sync.dma_start(bq2_sb[:], b_q2[:])
            nc.sync.dma_start(bk2_sb[:], b_k2[:])
            nc.sync.dma_start(bv1_sb[:], b_v1[:])
            nc.sync.dma_start(bv2_sb[:], b_v2[:])

            def load_w(wp):
                w_sb = p_w.tile([P, DT, D], BF16, tag="w")
                nc.sync.dma_start(w_sb[:], wp[:])
                return w_sb

            def proj_T(w_sb, src_sb, bias_sb, name):
                """out[e, n] = sum_d W[e,d] src[n,d] + b[e], e-major bf16."""
                o_sb = p_qk.tile([P, DT, N], BF16, tag="qk", name=name)
                for eo in range(DT):
                    for ch in range(NCH):
                        ps = p_ps.tile([P, CH], F32, tag="ps")
                        for do in range(DT):
                            nc.tensor.matmul(
                                ps[:],
                                w_sb[:, do, eo * P : (eo + 1) * P],
                                src_sb[:, do, ch * CH : (ch + 1) * CH],
                                start=(do == 0),
                                stop=(do == DT - 1),
                            )
                        nc.scalar.activation(
                            o_sb[:, eo, ch * CH : (ch + 1) * CH],
                            ps[:],
                            mybir.ActivationFunctionType.Identity,
                            bias=bias_sb[:, eo : eo + 1],
                        )
                return o_sb

            def proj_T_chunk(w_sb, src_sb, bias_sb, o_sb, ch):
                for eo in range(DT):
                    ps = p_ps.tile([P, CH], F32, tag="ps")
                    for do in range(DT):
                        nc.tensor.matmul(
                            ps[:],
                            w_sb[:, do, eo * P : (eo + 1) * P],
                            src_sb[:, do, ch * CH : (ch + 1) * CH],
                            start=(do == 0),
                            stop=(do == DT - 1),
                        )
                    nc.scalar.activation(
                        o_sb[:, eo, ch * CH : (ch + 1) * CH],
                        ps[:],
                        mybir.ActivationFunctionType.Identity,
                        bias=bias_sb[:, eo : eo + 1],
                    )

            def proj_V(w_sb, src_sb, bias_sb, name):
                """out[m, e] = sum_d src[m,d] W[e,d] + b[e], m-major fp8."""
                o_sb = p_v.tile([P, NT, D], F8, tag="v", name=name)
                for mt in range(NT):
                    ps = p_ps.tile([P, CH], F32, tag="ps")
                    for do in range(DT):
                        nc.tensor.matmul(
                            ps[:],
                            src_sb[:, do, mt * P : (mt + 1) * P],
                            w_sb[:, do, :],
                            start=(do == 0),
                            stop=(do == DT - 1),
                        )
                    nc.vector.tensor_add(out=o_sb[:, mt, :], in0=ps[:], in1=bias_sb[:])
                return o_sb

            def scores_chunk(kt_sb, qt_sb, e_dram, ch):
                """E[ch] = exp(scale * K^T Q) -> bf16 chunk -> DRAM (one DMA)."""
                HT = NT // 2
                for hf in range(2):
                    e_sb = p_est.tile([P, HT, CH], ar_dtype, tag="est")
                    for mi in range(HT):
                        mt = hf * HT + mi
                        ps = p_ps.tile([P, CH], F32, tag="ps")
                        for eo in range(DT):
                            nc.tensor.matmul(
                                ps[:],
                                kt_sb[:, eo, mt * P : (mt + 1) * P],
                                qt_sb[:, eo, ch * CH : (ch + 1) * CH],
                                start=(eo == 0),
                                stop=(eo == DT - 1),
                            )
                        nc.scalar.activation(
                            e_sb[:, mi, :], ps[:],
                            mybir.ActivationFunctionType.Exp,
                            scale=SCALE,
                        )
                    nc.sync.dma_start(
                        e_dram[ch, :, hf * HT : (hf + 1) * HT, :], e_sb[:]
                    )

            def ar_half(e_d, z_halves, h):
                nc.gpsimd.collective_compute(
                    "AllReduce",
                    mybir.AluOpType.add,
                    replica_groups=[list(range(NCORES))],
                    ins=[e_d[2 * h : 2 * h + 2].opt()],
                    outs=[z_halves[h][:].opt()],
                )

            # ==== phase A1/B1: K2 full, then Q1 chunks fused with E1 ====
            w_k2sb = load_w(w_k2)
            k2t = proj_T(w_k2sb, yt_sb, bk2_sb, "k2t")
            w_q1sb = load_w(w_q1)
            q1t = p_qk.tile([P, DT, N], BF16, tag="qk", name="q1t")
            for ch in range(NCH):
                proj_T_chunk(w_q1sb, xt_sb, bq1_sb, q1t, ch)
                scores_chunk(k2t, q1t, e1_d, ch)
                if ch == 1:
                    ar_half(e1_d, z1_h, 0)
            ar_half(e1_d, z1_h, 1)

            # ==== phase A2/B2: K1 full, then Q2 chunks fused with E2 ====
            w_k1sb = load_w(w_k1)
            k1t = proj_T(w_k1sb, xt_sb, bk1_sb, "k1t")
            w_q2sb = load_w(w_q2)
            q2t = p_qk.tile([P, DT, N], BF16, tag="qk", name="q2t")
            for ch in range(NCH):
                proj_T_chunk(w_q2sb, yt_sb, bq2_sb, q2t, ch)
                scores_chunk(k1t, q2t, e2_d, ch)
                if ch == 1:
                    ar_half(e2_d, [z2_h0], 0)
            ar_half(e2_d, [None, z2_h1], 1)

            # ======== phase A3: V projections (fp8 out), fills AR window ====
            w_sb = load_w(w_v2)
            v2 = proj_V(w_sb, yt_sb, bv2_sb, "v2")
            w_sb = load_w(w_v1)
            v1 = proj_V(w_sb, xt_sb, bv1_sb, "v1")

            # ======== phase C ========
            def make_ahat(e_d, z_src, ch, name):
                """Ahat[:, mt, c] = E/Z - 1/8 for chunk ch; fp8 [P, NT, CH]."""
                HT = NT // 2
                a_sb = p_ah.tile([P, NT, CH], F8, tag="ah", name=f"a{name}")
                for hf in range(2):
                    hsl = slice(hf * HT, (hf + 1) * HT)
                    eb = p_eb.tile([P, HT, CH], ar_dtype, tag="eb", name=f"eb{name}")
                    nc.sync.dma_start(eb[:], e_d[ch, :, hsl, :])
                    zb = p_zb.tile([P, HT, CH], ar_dtype, tag="zb", name=f"zb{name}")
                    nc.sync.dma_start(zb[:], z_src(ch)[:, hsl, :])
                    # T = E - 0.125*Z, in place into eb (one big 16-bit op)
                    nc.vector.scalar_tensor_tensor(
                        out=eb[:],
                        in0=zb[:],
                        scalar=-0.125,
                        in1=eb[:],
                        op0=mybir.AluOpType.mult,
                        op1=mybir.AluOpType.add,
                    )
                    for q in range(2):
                        msl = slice(q * 4, (q + 1) * 4)
                        asl = slice(hf * HT + q * 4, hf * HT + (q + 1) * 4)
                        zf = p_zf.tile([P, 4, CH], F32, tag="zf", name=f"zf{name}")
                        nc.scalar.activation(
                            zf[:], zb[:, msl, :],
                            mybir.ActivationFunctionType.Copy,
                        )
                        rz = p_zf.tile([P, 4, CH], F32, tag="zf", name=f"rz{name}")
                        nc.vector.reciprocal_approx_fast(out=rz[:], in_=zf[:])
                        nc.vector.tensor_mul(
                            out=a_sb[:, asl, :], in0=eb[:, msl, :], in1=rz[:]
                        )
                return a_sb

            def z1_src(ch):
                return z1_h[ch // 2][ch % 2]

            def z2_src(ch):
                return (z2_h0 if ch < 2 else z2_h1)[ch % 2]

            ps_held = {}

            def u1_pass(ch):
                a1 = make_ahat(e1_d, z1_src, ch, f"1{ch}")
                tiles = []
                for dt in range(DT):
                    dsl = slice(dt * P, (dt + 1) * P)
                    ps = p_ps.tile([P, CH], F32, tag="ps")
                    for tp in range(NT // 2):
                        nc.tensor.matmul(
                            ps[:],
                            v2[:, 2 * tp : 2 * tp + 2, dsl],
                            a1[:, 2 * tp : 2 * tp + 2, :],
                            start=(tp == 0),
                            stop=False,
                            perf_mode=mybir.MatmulPerfMode.DoubleRow,
                        )
                    tiles.append(ps)
                ps_held[ch] = tiles

            def u2_pass(ch):
                a2 = make_ahat(e2_d, z2_src, ch, f"2{ch}")
                for dt in range(DT):
                    dsl = slice(dt * P, (dt + 1) * P)
                    ps = ps_held[ch][dt]
                    for tp in range(NT // 2):
                        nc.tensor.matmul(
                            ps[:],
                            v1[:, 2 * tp : 2 * tp + 2, dsl],
                            a2[:, 2 * tp : 2 * tp + 2, :],
                            start=False,
                            stop=(tp == NT // 2 - 1),
                            perf_mode=mybir.MatmulPerfMode.DoubleRow,
                        )
                    xyres = p_small.tile([P, CH], F32, tag="xyres")
                    nc.scalar.dma_start(xyres[:], xyf[dt, ch])
                    ot = p_small.tile([P, CH], F32, tag="ot")
                    nc.vector.tensor_add(out=ot[:], in0=ps[:], in1=xyres[:])
                    nc.scalar.dma_start(out[dt, ch], ot[:])

            u1_pass(0)
            u1_pass(1)
            u2_pass(0)
            u1_pass(2)
            u2_pass(1)
            u1_pass(3)
            u2_pass(2)
            u2_pass(3)

    nc.compile()
    return nc


def _pmajor(a, inner):
    """[O*P, F] -> [P, O, F] partition-major."""
    o = a.shape[0] // inner
    return np.ascontiguousarray(a.reshape(o, inner, a.shape[1]).transpose(1, 0, 2))


def _blocked(a):
    """[D, N] -> [DT, NCH, P, CH] blocked."""
    return np.ascontiguousarray(a.reshape(DT, P, NCH, CH).transpose(0, 2, 1, 3))


def _prep_inputs(inputs):
    import ml_dtypes

    X = np.asarray(inputs["X"], dtype=np.float32)
    Y = np.asarray(inputs["Y"], dtype=np.float32)

    def wT(name):
        w = np.asarray(inputs[f"W_{name}"], dtype=np.float32)
        return _pmajor(w.T.astype(ml_dtypes.bfloat16), P)

    def bstripe(name):
        b = np.asarray(inputs[f"b_{name}"], dtype=np.float32)
        return np.ascontiguousarray(b.reshape(DT, P).T)

    def bbcast(name):
        b = np.asarray(inputs[f"b_{name}"], dtype=np.float32)
        return np.ascontiguousarray(np.broadcast_to(b, (P, D)))

    shared = {
        "WQ1T": wT("xq"),
        "WK1T": wT("xk"),
        "WV1T": wT("xv"),
        "WQ2T": wT("yq"),
        "WK2T": wT("yk"),
        "WV2T": wT("yv"),
        "BQ1": bstripe("xq"),
        "BK1": bstripe("xk"),
        "BQ2": bstripe("yq"),
        "BK2": bstripe("yk"),
        "BV1": bbcast("xv"),
        "BV2": bbcast("yv"),
    }
    # host-exact centered-A correction: (1/8) * colsum_m(V1[m,:] + V2[m,:])
    Wxv = np.asarray(inputs["W_xv"], np.float64)
    Wyv = np.asarray(inputs["W_yv"], np.float64)
    bxv = np.asarray(inputs["b_xv"], np.float64)
    byv = np.asarray(inputs["b_yv"], np.float64)
    xs = X.astype(np.float64).sum(axis=1)  # [B, D] colsum of X over tokens
    ys = Y.astype(np.float64).sum(axis=1)
    cs1 = xs @ Wxv.T + N * bxv  # [B, D] = colsum_m V1
    cs2 = ys @ Wyv.T + N * byv
    corr = ((cs1 + cs2) / 8.0).astype(np.float32)  # [B, D]

    in_maps = []
    for c in range(NCORES):
        xt = np.ascontiguousarray(X[c].T)
        yt = np.ascontiguousarray(Y[c].T)
        m = dict(shared)
        m["XYF"] = _blocked((xt + yt) + corr[c][:, None])
        m["XTB"] = _pmajor(xt.astype(ml_dtypes.bfloat16), P)
        m["YTB"] = _pmajor(yt.astype(ml_dtypes.bfloat16), P)
        in_maps.append(m)
    return in_maps


def _unblock(ot):
    """[DT, NCH, P, CH] -> [N, D] (transposed back)."""
    return ot.transpose(0, 2, 1, 3).reshape(D, N).T


def kernel(**inputs):
    if "nc" not in _CACHE:
        _CACHE["nc"] = build()
    nc = _CACHE["nc"]
    in_maps = _prep_inputs(inputs)
    res = run_bass_kernel_spmd(
        nc, in_maps, core_ids=list(range(NCORES)), **_CACHE.get("run_kwargs", {})
    )
    _CACHE["last_result"] = res
    out = np.stack(
        [np.ascontiguousarray(_unblock(res.results[c]["OT"])) for c in range(NCORES)]
    )
    return out.astype(np.float32)
